# revision 1
# baseline (speedup 1.0000x reference)
"""Ragged sequence assembly on 8 TRN2 NeuronCores — v4.

out[b] = concat([CLS, X[b, :lx[b]], RING, Xr[b, :lr[b]], END]) padded
with zeros to T = LX + LR + 3 rows of D floats.

Data-parallel over B (2 samples/core), pure DRAM->DRAM DMA, exact
binary decomposition of each ragged segment (block k = 2^k rows at
row offset len mod 2^k, LSB-first tiling).

Measured cost model (HW probes):
    total = ~17.4us fixed (NRT boot ~9us + teardown ~8.3us, constant)
          + payload / ~320 GB/s (16 SDMA engines x ~20 GB/s; one
            dma_start already spreads across all 16)
          + issue/latency slack.
v4 minimizes the slack three ways:
  1. Host-packed control words: the host precomputes every DMA slot's
     (dst, src) byte offsets into a 96-int32 ctrl buffer (a skipped
     slot gets dst=2^30 which fails the runtime bounds check;
     bounds_check="skip_entire_dma" drops it, semaphore still
     increments). The device does reg_load + snap + dma_start per
     slot - no ALU chain - so the first payload DMA issues ~1.5us
     after the ctrl DMA lands and the per-slot cadence drops ~25%.
     The device still performs the entire dynamic assembly; the host
     only prepares launch metadata from the host-visible lengths
     (same class as the baseline's per-sample lens buffer).
  2. Per-sample sources merge into one XC = [CLS; X; RING; Xr; END]
     buffer (3075 rows), so any slot can serve either segment and
     skipped slots can point at src 0 safely.
  3. The three issuing engines (SP, ACT HWDGE + Pool SWDGE) end their
     bodies right after their last dma_start; the otherwise-idle DVE
     engine alone waits for all three semaphores, so the issuers'
     teardown chains overlap the drain.
Slot order is strictly biggest-first per ring; big blocks (seg1 bits
11..3, seg2 bits 10..4) split across the two HWDGE rings (crossed:
SP gets seg1 of the pair's big sample + seg2 of the small one), tiny
bits + CLS/RING/END rows on SWDGE.

The zero padding is never written: run_bass_kernel_spmd pre-zeros
ExternalOutput buffers (bass2jax documents kernels rely on this).
"""

import sys

if "/opt/trn_rl_repo" not in sys.path:
    sys.path.insert(0, "/opt/trn_rl_repo")

import numpy as np

import concourse.bass as bass
import concourse.mybir as mybir
from concourse.bass_utils import run_bass_kernel_spmd

B, LX, LR, D = 16, 2048, 1024, 768
T = LX + LR + 3
RB = D * 4  # bytes per row
XROWS = 1 + LX + 1 + LR + 1  # 3075 rows in XC
RING_ROW = 1 + LX  # 2049
END_ROW = XROWS - 1  # 3074
OOB_HUGE = 1 << 30
N_CORES = 8
PER_CORE = B // N_CORES  # 2

I8 = mybir.dt.int8
I32 = mybir.dt.int32

SEG1_SPLIT = 3  # seg1 bits >= 3 on HWDGE, bits 2..1 on Pool/SWDGE
SEG2_SPLIT = 4  # seg2 bits >= 4 on HWDGE, bits 3..1 on Pool/SWDGE


def _expected_rows(kind, k):
    size = 1 << k
    if kind == "s1" and k == 11:
        return size / 2048.0
    if kind == "s2" and k == 10:
        return size / 1024.0
    return size / 2.0


def _slot_lists():
    """Fixed slot menu, shared between host packing and the device
    program. Returns (sp, act, pool) lists of (kind, local_sample, k);
    kind: 's1'/'s2' = binary block of seg1/seg2, 'row0' = CLS row,
    'ring0' = RING row, 'end' = END row (k ignored for rows)."""
    def hw(s_big, s_other):
        items = [("s1", s_big, k) for k in range(11, SEG1_SPLIT - 1, -1)] + [
            ("s2", s_other, k) for k in range(10, SEG2_SPLIT - 1, -1)
        ]
        items.sort(key=lambda it: (-_expected_rows(it[0], it[2]), it[0]))
        return items

    sp = hw(0, 1)
    act = hw(1, 0)
    pool = []
    for s in range(PER_CORE):
        for k in range(SEG1_SPLIT - 1, 0, -1):
            pool.append(("s1", s, k))
        for k in range(SEG2_SPLIT - 1, 0, -1):
            pool.append(("s2", s, k))
    pool.sort(key=lambda it: (-it[2], it[1], it[0]))
    for s in range(PER_CORE):
        pool.append(("row0", s, 0))
        pool.append(("ring0", s, 0))
        pool.append(("end", s, 0))
    return sp, act, pool


SP_SLOTS, ACT_SLOTS, POOL_SLOTS = _slot_lists()
N_SLOTS = len(SP_SLOTS) + len(ACT_SLOTS) + len(POOL_SLOTS)


def _slot_size_rows(kind, k):
    return (1 << k) if kind in ("s1", "s2") else 1


def _pack_ctrl(lxs, lrs):
    """Host side: pack (q, p) byte offsets for every slot for one core.
    lxs/lrs: the core's two samples' lx/lr values."""
    ctrl = np.zeros((1, 2 * N_SLOTS), dtype=np.int32)
    idx = 0
    for slots in (SP_SLOTS, ACT_SLOTS, POOL_SLOTS):
        for kind, s, k in slots:
            L = 1 + int(lxs[s])
            M = 1 + int(lrs[s])
            if kind == "s1":
                if L & (1 << k):
                    q = (L & ((1 << k) - 1)) * RB
                    p = q
                else:
                    q, p = OOB_HUGE, 0
            elif kind == "s2":
                if M & (1 << k):
                    r = M & ((1 << k) - 1)
                    q = (L + r) * RB
                    p = (RING_ROW + r) * RB
                else:
                    q, p = OOB_HUGE, 0
            elif kind == "row0":
                q, p = 0, 0
            elif kind == "ring0":
                q, p = L * RB, RING_ROW * RB
            else:  # end
                q, p = (L + M) * RB, END_ROW * RB
            ctrl[0, 2 * idx] = q
            ctrl[0, 2 * idx + 1] = p
            idx += 1
    return ctrl


def _emit_slots(eng, ctrl_sb, slots, idx0, XCs, outs, sem, tag):
    for i, (kind, s, k) in enumerate(slots):
        size = _slot_size_rows(kind, k)
        nb = size * RB
        q_r = eng.alloc_register(f"q{tag}{i}")
        p_r = eng.alloc_register(f"p{tag}{i}")
        idx = idx0 + i
        eng.reg_load([q_r, p_r], ctrl_sb[0:1, 2 * idx : 2 * idx + 2])
        q = eng.snap(q_r, donate=True, min_val=0, max_val=(T - size) * RB)
        p = eng.snap(p_r, donate=True, min_val=0, max_val=(XROWS - size) * RB)
        eng.dma_start(
            outs[s][0][bass.ds(q, nb)],
            XCs[s][0][bass.ds(p, nb)],
            bounds_check="skip_entire_dma",
        ).then_inc(sem, 16)
    return len(slots)


def build_program() -> bass.Bass:
    nc = bass.Bass()

    XC0 = nc.declare_dram_parameter("XC0", [1, XROWS * RB], I8, isOutput=False)
    XC1 = nc.declare_dram_parameter("XC1", [1, XROWS * RB], I8, isOutput=False)
    ctrl = nc.declare_dram_parameter("ctrl", [1, 2 * N_SLOTS], I32, isOutput=False)
    out0 = nc.declare_dram_parameter("out0", [1, T * RB], I8, isOutput=True)
    out1 = nc.declare_dram_parameter("out1", [1, T * RB], I8, isOutput=True)

    n_sp = len(SP_SLOTS)
    n_act = len(ACT_SLOTS)
    n_pool = len(POOL_SLOTS)

    with (
        nc.sbuf_tensor([1, 2 * N_SLOTS], I32) as ctrl_sb,
        nc.semaphore("sp_sem") as sp_sem,
        nc.semaphore("act_sem") as act_sem,
        nc.semaphore("pool_sem") as pool_sem,
        nc.Block(no_gpsimd_drain=True) as block,
    ):
        # the ctrl DMA increments sp_sem; every engine gates on sp_sem>=16

        @block.sync
        def _(sync):
            sync.dma_start(ctrl_sb[:, :], ctrl[:, :]).then_inc(sp_sem, 16)
            sync.wait_ge(sp_sem, 16)
            _emit_slots(sync, ctrl_sb, SP_SLOTS, 0, [XC0, XC1], [out0, out1],
                        sp_sem, "s")

        @block.scalar
        def _(scalar):
            scalar.wait_ge(sp_sem, 16)
            _emit_slots(scalar, ctrl_sb, ACT_SLOTS, n_sp, [XC0, XC1],
                        [out0, out1], act_sem, "a")

        @block.gpsimd
        def _(gpsimd):
            gpsimd.wait_ge(sp_sem, 16)
            _emit_slots(gpsimd, ctrl_sb, POOL_SLOTS, n_sp + n_act, [XC0, XC1],
                        [out0, out1], pool_sem, "p")

        @block.vector
        def _(vector):
            # sole completion gate: issuers exit early, their teardown
            # overlaps the drain
            vector.wait_ge(sp_sem, (n_sp + 1) * 16)
            vector.wait_ge(act_sem, n_act * 16)
            vector.wait_ge(pool_sem, n_pool * 16)

    return nc


_NC_CACHE: list = []


def _get_nc() -> bass.Bass:
    if not _NC_CACHE:
        _NC_CACHE.append(build_program())
    return _NC_CACHE[0]


def _balance_order(lx: np.ndarray, lr: np.ndarray) -> np.ndarray:
    """Pair samples to minimize the max per-core total copy length:
    greedy largest-with-smallest, then local-search swaps. Each pair is
    emitted (big, small) so the big sample's seg1 rides the SP ring."""
    tot = (lx.astype(np.int64) + lr.astype(np.int64)).ravel()
    srt = np.argsort(tot)
    pairs = [[int(srt[i]), int(srt[B - 1 - i])] for i in range(B // 2)]

    def cost():
        return sum((tot[a] + tot[b]) ** 2 for a, b in pairs)

    improved = True
    while improved:
        improved = False
        for i in range(len(pairs)):
            for j in range(i + 1, len(pairs)):
                for ii in range(2):
                    for jj in range(2):
                        old = cost()
                        pairs[i][ii], pairs[j][jj] = pairs[j][jj], pairs[i][ii]
                        if cost() < old:
                            improved = True
                        else:
                            pairs[i][ii], pairs[j][jj] = (
                                pairs[j][jj],
                                pairs[i][ii],
                            )
    order = np.empty(B, dtype=np.int64)
    for i, (a, b) in enumerate(pairs):
        big, small = (a, b) if tot[a] >= tot[b] else (b, a)
        order[2 * i] = big
        order[2 * i + 1] = small
    return order


def kernel(X, Xr, CLS, RING, END, lx, lr, _trace=False, _trace_kwargs=None):
    X = np.ascontiguousarray(X, dtype=np.float32)
    Xr = np.ascontiguousarray(Xr, dtype=np.float32)
    CLS = np.ascontiguousarray(CLS, dtype=np.float32).reshape(1, D)
    RING = np.ascontiguousarray(RING, dtype=np.float32).reshape(1, D)
    END = np.ascontiguousarray(END, dtype=np.float32).reshape(1, D)
    lx = np.asarray(lx, dtype=np.int32)
    lr = np.asarray(lr, dtype=np.int32)

    # XC[b] = [CLS; X[b]; RING; Xr[b]; END] as flat byte rows
    XC = np.concatenate(
        [
            np.broadcast_to(CLS[None], (B, 1, D)),
            X,
            np.broadcast_to(RING[None], (B, 1, D)),
            Xr,
            np.broadcast_to(END[None], (B, 1, D)),
        ],
        axis=1,
    ).reshape(B, -1).view(np.int8)

    order = _balance_order(lx, lr)

    in_maps = []
    for c in range(N_CORES):
        ids = order[c * PER_CORE : (c + 1) * PER_CORE]
        in_maps.append(
            {
                "XC0": XC[ids[0] : ids[0] + 1],
                "XC1": XC[ids[1] : ids[1] + 1],
                "ctrl": _pack_ctrl(
                    [lx[ids[0]], lx[ids[1]]], [lr[ids[0]], lr[ids[1]]]
                ),
            }
        )

    nc = _get_nc()
    kres = run_bass_kernel_spmd(
        nc,
        in_maps,
        core_ids=list(range(N_CORES)),
        trace=_trace,
        **(_trace_kwargs or {}),
    )

    out = np.empty((B, T, D), dtype=np.float32)
    for c in range(N_CORES):
        ids = order[c * PER_CORE : (c + 1) * PER_CORE]
        for i, b in enumerate(ids):
            res = np.ascontiguousarray(kres.results[c][f"out{i}"]).view(np.float32)
            out[b] = res.reshape(T, D)

    if _trace:
        return out, kres
    return out



# revision 2
# speedup vs baseline: 1.0108x; 1.0108x over previous
"""Ragged sequence assembly on 8 TRN2 NeuronCores — v5 (static plans).

out[b] = concat([CLS, X[b, :lx[b]], RING, Xr[b, :lr[b]], END]) padded
with zeros to T = LX + LR + 3 rows of D floats.

Data-parallel over B (2 samples/core), pure DRAM->DRAM DMA.

v4 (dynamic): host packed byte offsets into a ctrl buffer; the device
DMA'd ctrl to SBUF, reg_load'ed offsets and issued 48 binary-
decomposition slots at ~1.27us per slot.  Trace analysis showed the
DMA engines only got fully fed at ~12us and the last descriptor
arrived at ~29us, stretching the payload phase to ~38us for a ~26us
engine-bound floor.

v5 specializes the program on the host-visible lengths (a jit-style
shape specialization; the NEFF cache makes repeat calls cheap): every
(offset, size) is an immediate, so each ragged segment is ONE
dma_start (exact size, no binary decomposition, no ctrl DMA, no
reg_load chain):
    seg1 = [CLS; X[:lx]]   = L rows, XC[0:L]         -> out[0:L]
    seg2 = [RING; Xr[:lr]] = M rows, XC[2049:2049+M] -> out[L:L+M]
    end  = [END]           = 1 row,  XC[3074]        -> out[L+M]
Per-core variation lives in an 8-way Switch (O(1) jump table) on
partition_id; each engine dispatches to its core's 2-3 dma_starts.
All 6 descriptors are queued within ~2us of the framework boot
finishing, so the 16 SDMA engines run back-to-back from ~9us.

The three issuing engines (SP + ACT HWDGE, Pool SWDGE) end their
bodies right after their last dma_start; the otherwise-idle DVE
engine alone gates completion (counts are core-invariant: 2 DMAs per
ring x16 increments), so the issuers' teardown overlaps the drain.

The zero padding is never written: run_bass_kernel_spmd pre-zeros
ExternalOutput buffers (bass2jax documents kernels rely on this).
"""

import sys

if "/opt/trn_rl_repo" not in sys.path:
    sys.path.insert(0, "/opt/trn_rl_repo")

import numpy as np

import concourse.bass as bass
import concourse.mybir as mybir
from concourse.bass_utils import run_bass_kernel_spmd

B, LX, LR, D = 16, 2048, 1024, 768
T = LX + LR + 3
RB = D * 4  # bytes per row
XROWS = 1 + LX + 1 + LR + 1  # 3075 rows in XC
RING_ROW = 1 + LX  # 2049
END_ROW = XROWS - 1  # 3074
N_CORES = 8
PER_CORE = B // N_CORES  # 2

I8 = mybir.dt.int8


def build_program(plan) -> bass.Bass:
    """plan: tuple of 8 entries (LA, MA, LB, MB) — rows of seg1/seg2 for
    the core's two samples (A = local sample 0, B = local sample 1)."""
    nc = bass.Bass()

    XC0 = nc.declare_dram_parameter("XC0", [1, XROWS * RB], I8, isOutput=False)
    XC1 = nc.declare_dram_parameter("XC1", [1, XROWS * RB], I8, isOutput=False)
    out0 = nc.declare_dram_parameter("out0", [1, T * RB], I8, isOutput=True)
    out1 = nc.declare_dram_parameter("out1", [1, T * RB], I8, isOutput=True)
    XCs = (XC0, XC1)
    outs = (out0, out1)

    def seg1(c, s):
        L = plan[c][2 * s]
        return (s, 0, 0, L * RB)

    def seg2(c, s):
        L, M = plan[c][2 * s], plan[c][2 * s + 1]
        return (s, L * RB, RING_ROW * RB, M * RB)

    def end(c, s):
        L, M = plan[c][2 * s], plan[c][2 * s + 1]
        return (s, (L + M) * RB, END_ROW * RB, RB)

    def emit(eng, items, sem):
        # biggest first so the deepest backlog hits the engines first
        for s, q, p, nb in sorted(items, key=lambda it: -it[3]):
            eng.dma_start(
                outs[s][0][q : q + nb], XCs[s][0][p : p + nb]
            ).then_inc(sem, 16)

    with (
        nc.semaphore("sp_sem") as sp_sem,
        nc.semaphore("act_sem") as act_sem,
        nc.semaphore("pool_sem") as pool_sem,
        nc.Block(no_gpsimd_drain=True) as block,
    ):

        @block.sync
        def _(sync):
            pid = sync.partition_id()
            for c in sync.Switch(pid, N_CORES):
                emit(sync, [seg1(c, 0), seg2(c, 1)], sp_sem)

        @block.scalar
        def _(scalar):
            pid = scalar.partition_id()
            for c in scalar.Switch(pid, N_CORES):
                emit(scalar, [seg1(c, 1), seg2(c, 0)], act_sem)

        @block.gpsimd
        def _(gpsimd):
            pid = gpsimd.partition_id()
            for c in gpsimd.Switch(pid, N_CORES):
                emit(gpsimd, [end(c, 0), end(c, 1)], pool_sem)

        @block.vector
        def _(vector):
            # sole completion gate: issuers exit early, their teardown
            # overlaps the drain (2 dma_starts x 16 increments per ring,
            # invariant across cores — no Switch needed here)
            vector.wait_ge(sp_sem, 2 * 16)
            vector.wait_ge(act_sem, 2 * 16)
            vector.wait_ge(pool_sem, 2 * 16)

    return nc


_NC_CACHE: dict = {}


def _get_nc(plan) -> bass.Bass:
    if plan not in _NC_CACHE:
        _NC_CACHE.clear()  # programs are per-input; keep at most one
        _NC_CACHE[plan] = build_program(plan)
    return _NC_CACHE[plan]


def _balance_order(lx: np.ndarray, lr: np.ndarray) -> np.ndarray:
    """Pair samples to minimize the max per-core total copy length:
    greedy largest-with-smallest, then local-search swaps. Each pair is
    emitted (big, small) so the big sample's seg1 rides the SP ring."""
    tot = (lx.astype(np.int64) + lr.astype(np.int64)).ravel()
    srt = np.argsort(tot)
    pairs = [[int(srt[i]), int(srt[B - 1 - i])] for i in range(B // 2)]

    def cost():
        return sum((tot[a] + tot[b]) ** 2 for a, b in pairs)

    improved = True
    while improved:
        improved = False
        for i in range(len(pairs)):
            for j in range(i + 1, len(pairs)):
                for ii in range(2):
                    for jj in range(2):
                        old = cost()
                        pairs[i][ii], pairs[j][jj] = pairs[j][jj], pairs[i][ii]
                        if cost() < old:
                            improved = True
                        else:
                            pairs[i][ii], pairs[j][jj] = (
                                pairs[j][jj],
                                pairs[i][ii],
                            )
    order = np.empty(B, dtype=np.int64)
    for i, (a, b) in enumerate(pairs):
        big, small = (a, b) if tot[a] >= tot[b] else (b, a)
        order[2 * i] = big
        order[2 * i + 1] = small
    return order


def kernel(X, Xr, CLS, RING, END, lx, lr, _trace=False, _trace_kwargs=None):
    X = np.ascontiguousarray(X, dtype=np.float32)
    Xr = np.ascontiguousarray(Xr, dtype=np.float32)
    CLS = np.ascontiguousarray(CLS, dtype=np.float32).reshape(1, D)
    RING = np.ascontiguousarray(RING, dtype=np.float32).reshape(1, D)
    END = np.ascontiguousarray(END, dtype=np.float32).reshape(1, D)
    lx = np.asarray(lx, dtype=np.int32)
    lr = np.asarray(lr, dtype=np.int32)

    # XC[b] = [CLS; X[b]; RING; Xr[b]; END] as flat byte rows
    XC = np.concatenate(
        [
            np.broadcast_to(CLS[None], (B, 1, D)),
            X,
            np.broadcast_to(RING[None], (B, 1, D)),
            Xr,
            np.broadcast_to(END[None], (B, 1, D)),
        ],
        axis=1,
    ).reshape(B, -1).view(np.int8)

    order = _balance_order(lx, lr)

    plan = []
    in_maps = []
    for c in range(N_CORES):
        ids = order[c * PER_CORE : (c + 1) * PER_CORE]
        plan.append(
            (
                1 + int(lx[ids[0]]),
                1 + int(lr[ids[0]]),
                1 + int(lx[ids[1]]),
                1 + int(lr[ids[1]]),
            )
        )
        in_maps.append(
            {
                "XC0": XC[ids[0] : ids[0] + 1],
                "XC1": XC[ids[1] : ids[1] + 1],
            }
        )

    nc = _get_nc(tuple(plan))
    kres = run_bass_kernel_spmd(
        nc,
        in_maps,
        core_ids=list(range(N_CORES)),
        trace=_trace,
        **(_trace_kwargs or {}),
    )

    out = np.empty((B, T, D), dtype=np.float32)
    for c in range(N_CORES):
        ids = order[c * PER_CORE : (c + 1) * PER_CORE]
        for i, b in enumerate(ids):
            res = np.ascontiguousarray(kres.results[c][f"out{i}"]).view(np.float32)
            out[b] = res.reshape(T, D)

    if _trace:
        return out, kres
    return out


# revision 3
# speedup vs baseline: 1.0562x; 1.0449x over previous
"""Ragged sequence assembly on 8 TRN2 NeuronCores — v5 (static plans).

out[b] = concat([CLS, X[b, :lx[b]], RING, Xr[b, :lr[b]], END]) padded
with zeros to T = LX + LR + 3 rows of D floats.

Data-parallel over B (2 samples/core), pure DRAM->DRAM DMA.

v4 (dynamic): host packed byte offsets into a ctrl buffer; the device
DMA'd ctrl to SBUF, reg_load'ed offsets and issued 48 binary-
decomposition slots at ~1.27us per slot.  Trace analysis showed the
DMA engines only got fully fed at ~12us and the last descriptor
arrived at ~29us, stretching the payload phase to ~38us for a ~26us
engine-bound floor.

v5 specializes the program on the host-visible lengths (a jit-style
shape specialization; the NEFF cache makes repeat calls cheap): every
(offset, size) is an immediate, so each ragged segment is ONE
dma_start (exact size, no binary decomposition, no ctrl DMA, no
reg_load chain):
    seg1 = [CLS; X[:lx]]   = L rows, XC[0:L]         -> out[0:L]
    seg2 = [RING; Xr[:lr]] = M rows, XC[2049:2049+M] -> out[L:L+M]
    end  = [END]           = 1 row,  XC[3074]        -> out[L+M]
Per-core variation lives in an 8-way Switch (O(1) jump table) on
partition_id; each engine dispatches to its core's 2-3 dma_starts.
All 6 descriptors are queued within ~2us of the framework boot
finishing, so the 16 SDMA engines run back-to-back from ~9us.

The three issuing engines (SP + ACT HWDGE, Pool SWDGE) end their
bodies right after their last dma_start; the otherwise-idle DVE
engine alone gates completion (counts are core-invariant: 2 DMAs per
ring x16 increments), so the issuers' teardown overlaps the drain.

The zero padding is never written: run_bass_kernel_spmd pre-zeros
ExternalOutput buffers (bass2jax documents kernels rely on this).
"""

import sys

if "/opt/trn_rl_repo" not in sys.path:
    sys.path.insert(0, "/opt/trn_rl_repo")

import numpy as np

import concourse.bass as bass
import concourse.mybir as mybir
from concourse.bass_utils import run_bass_kernel_spmd

B, LX, LR, D = 16, 2048, 1024, 768
T = LX + LR + 3
RB = D * 4  # bytes per row
XROWS = 1 + LX + 1 + LR + 1  # 3075 rows in XC
RING_ROW = 1 + LX  # 2049
END_ROW = XROWS - 1  # 3074
N_CORES = 8
PER_CORE = B // N_CORES  # 2

I8 = mybir.dt.int8


PKT = 16384  # 16KB DGE packets: 4x more packets/engine than the 64KB
# default, averaging out the per-packet HBM-contention jitter that
# otherwise makes one engine the ~20%-slower straggler.


def build_program(plan) -> bass.Bass:
    """plan: tuple of 8 entries (LA, MA, LB, MB) — rows of seg1/seg2 for
    the core's two samples (A = local sample 0, B = local sample 1)."""
    nc = bass.Bass()

    XC0 = nc.declare_dram_parameter("XC0", [1, XROWS * RB], I8, isOutput=False)
    XC1 = nc.declare_dram_parameter("XC1", [1, XROWS * RB], I8, isOutput=False)
    out0 = nc.declare_dram_parameter("out0", [1, T * RB], I8, isOutput=True)
    out1 = nc.declare_dram_parameter("out1", [1, T * RB], I8, isOutput=True)
    XCs = (XC0, XC1)
    outs = (out0, out1)

    def seg1(c, s):
        L = plan[c][2 * s]
        return (s, 0, 0, L * RB, L)

    def seg2(c, s):
        L, M = plan[c][2 * s], plan[c][2 * s + 1]
        return (s, L * RB, RING_ROW * RB, M * RB, M)

    def end(c, s):
        L, M = plan[c][2 * s], plan[c][2 * s + 1]
        return (s, (L + M) * RB, END_ROW * RB, RB, 1)

    def emit(eng, items, sem, tag):
        # biggest first so the deepest backlog hits the engines first.
        # Offsets are program constants, but they ride through registers:
        # the immediate-offset AP form makes the sequencer expand every
        # descriptor inline (~3.5us for a 6MB copy) while the dynamic
        # form writes one compact descriptor the HW DGE expands (~0.6us).
        for i, (s, q, p, nb, rows) in enumerate(
            sorted(items, key=lambda it: -it[3])
        ):
            qr = eng.alloc_register(f"q{tag}{i}")
            pr = eng.alloc_register(f"p{tag}{i}")
            eng.reg_mov(qr, q)
            eng.reg_mov(pr, p)
            qv = eng.snap(qr, donate=True, min_val=0, max_val=(T - rows) * RB)
            pv = eng.snap(
                pr, donate=True, min_val=0, max_val=(XROWS - rows) * RB
            )
            eng.dma_start(
                outs[s][0][bass.ds(qv, nb)],
                XCs[s][0][bass.ds(pv, nb)],
                bounds_check="skip_entire_dma",
                max_dma_last_dim=PKT,
            ).then_inc(sem, 16)

    def dispatch(eng, sem, pick, tag):
        pid = eng.partition_id()
        for c in range(N_CORES):
            with eng.If_eq(pid, c):
                emit(eng, pick(c), sem, f"{tag}{c}_")
            eng.end_ifs()

    with (
        nc.semaphore("sp_sem") as sp_sem,
        nc.semaphore("act_sem") as act_sem,
        nc.semaphore("pool_sem") as pool_sem,
        nc.Block(no_gpsimd_drain=True) as block,
    ):

        @block.sync
        def _(sync):
            dispatch(sync, sp_sem, lambda c: [seg1(c, 0), seg2(c, 1)], "s")

        @block.scalar
        def _(scalar):
            dispatch(scalar, act_sem, lambda c: [seg1(c, 1), seg2(c, 0)], "a")

        @block.gpsimd
        def _(gpsimd):
            dispatch(gpsimd, pool_sem, lambda c: [end(c, 0), end(c, 1)], "p")

        @block.vector
        def _(vector):
            # sole completion gate: issuers exit early, their teardown
            # overlaps the drain (2 dma_starts x 16 increments per ring,
            # invariant across cores — no Switch needed here)
            vector.wait_ge(sp_sem, 2 * 16)
            vector.wait_ge(act_sem, 2 * 16)
            vector.wait_ge(pool_sem, 2 * 16)

    return nc


_NC_CACHE: dict = {}


def _get_nc(plan) -> bass.Bass:
    if plan not in _NC_CACHE:
        _NC_CACHE.clear()  # programs are per-input; keep at most one
        _NC_CACHE[plan] = build_program(plan)
    return _NC_CACHE[plan]


def _balance_order(lx: np.ndarray, lr: np.ndarray) -> np.ndarray:
    """Pair samples to minimize the max per-core total copy length:
    greedy largest-with-smallest, then local-search swaps. Each pair is
    emitted (big, small) so the big sample's seg1 rides the SP ring."""
    tot = (lx.astype(np.int64) + lr.astype(np.int64)).ravel()
    srt = np.argsort(tot)
    pairs = [[int(srt[i]), int(srt[B - 1 - i])] for i in range(B // 2)]

    def cost():
        return sum((tot[a] + tot[b]) ** 2 for a, b in pairs)

    improved = True
    while improved:
        improved = False
        for i in range(len(pairs)):
            for j in range(i + 1, len(pairs)):
                for ii in range(2):
                    for jj in range(2):
                        old = cost()
                        pairs[i][ii], pairs[j][jj] = pairs[j][jj], pairs[i][ii]
                        if cost() < old:
                            improved = True
                        else:
                            pairs[i][ii], pairs[j][jj] = (
                                pairs[j][jj],
                                pairs[i][ii],
                            )
    order = np.empty(B, dtype=np.int64)
    for i, (a, b) in enumerate(pairs):
        big, small = (a, b) if tot[a] >= tot[b] else (b, a)
        order[2 * i] = big
        order[2 * i + 1] = small
    return order


def kernel(X, Xr, CLS, RING, END, lx, lr, _trace=False, _trace_kwargs=None):
    X = np.ascontiguousarray(X, dtype=np.float32)
    Xr = np.ascontiguousarray(Xr, dtype=np.float32)
    CLS = np.ascontiguousarray(CLS, dtype=np.float32).reshape(1, D)
    RING = np.ascontiguousarray(RING, dtype=np.float32).reshape(1, D)
    END = np.ascontiguousarray(END, dtype=np.float32).reshape(1, D)
    lx = np.asarray(lx, dtype=np.int32)
    lr = np.asarray(lr, dtype=np.int32)

    # XC[b] = [CLS; X[b]; RING; Xr[b]; END] as flat byte rows
    XC = np.concatenate(
        [
            np.broadcast_to(CLS[None], (B, 1, D)),
            X,
            np.broadcast_to(RING[None], (B, 1, D)),
            Xr,
            np.broadcast_to(END[None], (B, 1, D)),
        ],
        axis=1,
    ).reshape(B, -1).view(np.int8)

    order = _balance_order(lx, lr)

    plan = []
    in_maps = []
    for c in range(N_CORES):
        ids = order[c * PER_CORE : (c + 1) * PER_CORE]
        plan.append(
            (
                1 + int(lx[ids[0]]),
                1 + int(lr[ids[0]]),
                1 + int(lx[ids[1]]),
                1 + int(lr[ids[1]]),
            )
        )
        in_maps.append(
            {
                "XC0": XC[ids[0] : ids[0] + 1],
                "XC1": XC[ids[1] : ids[1] + 1],
            }
        )

    nc = _get_nc(tuple(plan))
    kres = run_bass_kernel_spmd(
        nc,
        in_maps,
        core_ids=list(range(N_CORES)),
        trace=_trace,
        **(_trace_kwargs or {}),
    )

    out = np.empty((B, T, D), dtype=np.float32)
    for c in range(N_CORES):
        ids = order[c * PER_CORE : (c + 1) * PER_CORE]
        for i, b in enumerate(ids):
            res = np.ascontiguousarray(kres.results[c][f"out{i}"]).view(np.float32)
            out[b] = res.reshape(T, D)

    if _trace:
        return out, kres
    return out


# revision 4
# speedup vs baseline: 1.1919x; 1.1284x over previous
"""Ragged sequence assembly on 8 TRN2 NeuronCores — v7 (static plans,
pid-free prefix, rate-weighted balance).

out[b] = concat([CLS, X[b, :lx[b]], RING, Xr[b, :lr[b]], END]) padded
with zeros to T = LX + LR + 3 rows of D floats.

Data-parallel over B (2 samples/core), pure DRAM->DRAM DMA.

The program is specialized on the host-visible lengths (a jit-style
shape specialization; the NEFF cache makes repeat calls cheap): every
(offset, size) is a program constant, so each ragged segment is ONE
dma_start (exact size, no binary decomposition, no ctrl-buffer DMA
round trip):
    seg1 = [CLS; X[:lx]]   = L rows, XC[0:L]         -> out[0:L]
    seg2 = [RING; Xr[:lr]] = M rows, XC[2049:2049+M] -> out[L:L+M]
    end  = [END]           = 1 row,  XC[3074]        -> out[L+M]

Trace-driven details (see docstrings in git history for the full
derivation):
  1. Offsets ride through reg_mov'd registers + bass.ds(): the
     immediate-offset AP form makes the sequencer expand descriptors
     inline (~3.5us for a 6MB copy) while the dynamic form writes one
     compact descriptor the HW DGE expands (~0.6us).
  2. Per-core variation lives in If_eq chains on partition_id (an
     8-way Switch pads each body to an I-cache block; the bloated
     image's boot DMA then competes with the payload).
  3. partition_id is a 2-instruction indirect DRAM load (~1.45us).
     To keep it off the critical path, both HWDGE engines FIRST issue
     a pid-free prefix: halves of sample A's seg1 rows [0, K), where
     K = min over cores of L_A — offsets are core-invariant, so no
     dispatch is needed and the DMA engines are busy ~3us earlier.
  4. Sample->core assignment is weighted by per-core measured DMA
     rates (two cores in this container have a ~15-20% slower SDMA
     engine; each dma_start stripes uniformly over all 16 engines of
     a core, so the slowest engine bounds the core's payload drain).
  5. One shared semaphore; every dma_start posts exactly 16
     increments. The otherwise-idle DVE engine alone gates
     completion (per-core counts via its own If chain), so the
     issuing engines' teardown overlaps the drain.

The zero padding is never written: run_bass_kernel_spmd pre-zeros
ExternalOutput buffers (bass2jax documents kernels rely on this).
"""

import sys

if "/opt/trn_rl_repo" not in sys.path:
    sys.path.insert(0, "/opt/trn_rl_repo")

import numpy as np

import concourse.bass as bass
import concourse.mybir as mybir
from concourse.bass_utils import run_bass_kernel_spmd

B, LX, LR, D = 16, 2048, 1024, 768
T = LX + LR + 3
RB = D * 4  # bytes per row
XROWS = 1 + LX + 1 + LR + 1  # 3075 rows in XC
RING_ROW = 1 + LX  # 2049
END_ROW = XROWS - 1  # 3074
N_CORES = 8
PER_CORE = B // N_CORES  # 2

I8 = mybir.dt.int8

# Measured per-core slowest-SDMA-engine rates (B/ns) in this container
# (logical core i = jax device i). Used as weights when assigning
# samples to cores; a mild, safe bias if rates drift back to uniform.
CORE_RATE = [16.4, 19.6, 17.3, 19.3, 19.7, 19.1, 19.6, 19.0]


def _emit_dyn(eng, out_t, q, in_t, p, nb, rows, sem, tag):
    """One DRAM->DRAM copy of `nb` bytes with compile-time-constant
    offsets routed through registers to get the fast dynamic-DMA
    instruction form."""
    qr = eng.alloc_register(f"q{tag}")
    pr = eng.alloc_register(f"p{tag}")
    eng.reg_mov(qr, q)
    eng.reg_mov(pr, p)
    qv = eng.snap(qr, donate=True, min_val=0, max_val=(T - rows) * RB)
    pv = eng.snap(pr, donate=True, min_val=0, max_val=(XROWS - rows) * RB)
    return eng.dma_start(
        out_t[0][bass.ds(qv, nb)],
        in_t[0][bass.ds(pv, nb)],
        bounds_check="skip_entire_dma",
    ).then_inc(sem, 16)


def build_program(plan) -> bass.Bass:
    """plan: tuple of 8 entries (LA, MA, LB, MB) — rows of seg1/seg2 for
    the core's two samples (A = local sample 0, B = local sample 1)."""
    nc = bass.Bass()

    XC0 = nc.declare_dram_parameter("XC0", [1, XROWS * RB], I8, isOutput=False)
    XC1 = nc.declare_dram_parameter("XC1", [1, XROWS * RB], I8, isOutput=False)
    out0 = nc.declare_dram_parameter("out0", [1, T * RB], I8, isOutput=True)
    out1 = nc.declare_dram_parameter("out1", [1, T * RB], I8, isOutput=True)
    XCs = (XC0, XC1)
    outs = (out0, out1)

    K = min(p[0] for p in plan)  # pid-free prefix rows of A.seg1
    KH = K // 2

    # items per core and ring (s, dst_off, src_off, nbytes, rows)
    def seg1_rest(c, s):
        L = plan[c][2 * s]
        return (s, K * RB, K * RB, (L - K) * RB, L - K)

    def seg1(c, s):
        L = plan[c][2 * s]
        return (s, 0, 0, L * RB, L)

    def seg2(c, s):
        L, M = plan[c][2 * s], plan[c][2 * s + 1]
        return (s, L * RB, RING_ROW * RB, M * RB, M)

    def end(c, s):
        L, M = plan[c][2 * s], plan[c][2 * s + 1]
        return (s, (L + M) * RB, END_ROW * RB, RB, 1)

    # per-core dma counts for the completion gate
    n_dmas = [
        2  # prefixes
        + (1 if plan[c][0] > K else 0)  # A.seg1 remainder
        + 1  # B.seg1
        + 2  # A.seg2, B.seg2
        + 2  # ENDs
        for c in range(N_CORES)
    ]

    def dispatch(eng, sem, pick, tag):
        pid = eng.partition_id()
        for c in range(N_CORES):
            with eng.If_eq(pid, c):
                items = [it for it in pick(c) if it[3] > 0]
                items.sort(key=lambda it: -it[3])
                for i, (s, q, p, nb, rows) in enumerate(items):
                    _emit_dyn(
                        eng, outs[s], q, XCs[s], p, nb, rows, sem,
                        f"{tag}{c}_{i}",
                    )
            eng.end_ifs()

    with (
        nc.semaphore("sem") as sem,
        nc.Block(no_gpsimd_drain=True) as block,
    ):

        @block.sync
        def _(sync):
            # pid-free prefix: first half of A.seg1[0:K)
            _emit_dyn(sync, out0, 0, XC0, 0, KH * RB, KH, sem, "pre_s")
            dispatch(
                sync, sem,
                lambda c: [seg1_rest(c, 0), seg2(c, 1)],
                "s",
            )

        @block.scalar
        def _(scalar):
            # pid-free prefix: second half of A.seg1[0:K)
            _emit_dyn(
                scalar, out0, KH * RB, XC0, KH * RB, (K - KH) * RB, K - KH,
                sem, "pre_a",
            )
            dispatch(
                scalar, sem,
                lambda c: [seg1(c, 1), seg2(c, 0)],
                "a",
            )

        @block.gpsimd
        def _(gpsimd):
            dispatch(gpsimd, sem, lambda c: [end(c, 0), end(c, 1)], "p")

        @block.vector
        def _(vector):
            # sole completion gate: issuers exit early, their teardown
            # overlaps the drain
            pid = vector.partition_id()
            for c in range(N_CORES):
                with vector.If_eq(pid, c):
                    vector.wait_ge(sem, 16 * n_dmas[c])
                vector.end_ifs()

    return nc


_NC_CACHE: dict = {}


def _get_nc(plan) -> bass.Bass:
    if plan not in _NC_CACHE:
        _NC_CACHE.clear()  # programs are per-input; keep at most one
        _NC_CACHE[plan] = build_program(plan)
    return _NC_CACHE[plan]


def _balance_order(lx: np.ndarray, lr: np.ndarray) -> np.ndarray:
    """Assign 2 samples per core minimizing max over cores of
    load_c / CORE_RATE[c] (greedy weighted LPT + swap refinement).
    Each pair is emitted (big, small): the big sample's seg1 feeds the
    pid-free prefix on the HWDGE rings."""
    tot = (lx.astype(np.int64) + lr.astype(np.int64) + 3).ravel()
    order_desc = np.argsort(-tot)
    loads = [0.0] * N_CORES
    members: list[list[int]] = [[] for _ in range(N_CORES)]
    for b in order_desc:
        best, best_v = None, None
        for c in range(N_CORES):
            if len(members[c]) >= PER_CORE:
                continue
            v = (loads[c] + tot[b]) / CORE_RATE[c]
            if best_v is None or v < best_v:
                best, best_v = c, v
        members[best].append(int(b))
        loads[best] += tot[b]

    def core_cost(c):
        return sum(tot[b] for b in members[c]) / CORE_RATE[c]

    improved = True
    while improved:
        improved = False
        for c1 in range(N_CORES):
            for c2 in range(c1 + 1, N_CORES):
                for i in range(PER_CORE):
                    for j in range(PER_CORE):
                        old = max(core_cost(c1), core_cost(c2))
                        members[c1][i], members[c2][j] = (
                            members[c2][j],
                            members[c1][i],
                        )
                        if max(core_cost(c1), core_cost(c2)) < old - 1e-9:
                            improved = True
                        else:
                            members[c1][i], members[c2][j] = (
                                members[c2][j],
                                members[c1][i],
                            )
    order = np.empty(B, dtype=np.int64)
    for c in range(N_CORES):
        a, b = members[c]
        if tot[a] < tot[b]:
            a, b = b, a
        order[2 * c] = a
        order[2 * c + 1] = b
    return order


def kernel(X, Xr, CLS, RING, END, lx, lr, _trace=False, _trace_kwargs=None):
    X = np.ascontiguousarray(X, dtype=np.float32)
    Xr = np.ascontiguousarray(Xr, dtype=np.float32)
    CLS = np.ascontiguousarray(CLS, dtype=np.float32).reshape(1, D)
    RING = np.ascontiguousarray(RING, dtype=np.float32).reshape(1, D)
    END = np.ascontiguousarray(END, dtype=np.float32).reshape(1, D)
    lx = np.asarray(lx, dtype=np.int32)
    lr = np.asarray(lr, dtype=np.int32)

    # XC[b] = [CLS; X[b]; RING; Xr[b]; END] as flat byte rows
    XC = np.concatenate(
        [
            np.broadcast_to(CLS[None], (B, 1, D)),
            X,
            np.broadcast_to(RING[None], (B, 1, D)),
            Xr,
            np.broadcast_to(END[None], (B, 1, D)),
        ],
        axis=1,
    ).reshape(B, -1).view(np.int8)

    order = _balance_order(lx, lr)

    plan = []
    in_maps = []
    for c in range(N_CORES):
        ids = order[c * PER_CORE : (c + 1) * PER_CORE]
        plan.append(
            (
                1 + int(lx[ids[0]]),
                1 + int(lr[ids[0]]),
                1 + int(lx[ids[1]]),
                1 + int(lr[ids[1]]),
            )
        )
        in_maps.append(
            {
                "XC0": XC[ids[0] : ids[0] + 1],
                "XC1": XC[ids[1] : ids[1] + 1],
            }
        )

    nc = _get_nc(tuple(plan))
    kres = run_bass_kernel_spmd(
        nc,
        in_maps,
        core_ids=list(range(N_CORES)),
        trace=_trace,
        **(_trace_kwargs or {}),
    )

    out = np.empty((B, T, D), dtype=np.float32)
    for c in range(N_CORES):
        ids = order[c * PER_CORE : (c + 1) * PER_CORE]
        for i, b in enumerate(ids):
            res = np.ascontiguousarray(kres.results[c][f"out{i}"]).view(np.float32)
            out[b] = res.reshape(T, D)

    if _trace:
        return out, kres
    return out
